# revision 32
# baseline (speedup 1.0000x reference)
"""Quantized 3x3 conv (8-bit symmetric STE quantization of x and w, then
stride-1 pad-1 conv) on 8 Trainium2 NeuronCores.

Strategy
--------
Data-parallel over batch: 4 images per core (32/8).  Per core:
  * x is quantized on-device to integers kx in [-127,127] stored as bf16
    (exact), via 3 elementwise passes:
      P0 (DVE):  t = min(x * s, 127.25)            s = 1/step  (fp32)
      P1 (DVE):  v = max(t, -127.25) + 1.5*2^23    (magic round-half-even)
      P2 (ACT):  k = v - 1.5*2^23  -> bf16          (exact; relayout to a
                                                     58-wide zero-padded grid)
    This reproduces jnp.round(x/step) bit-exactly (verified vs the fp32
    reference on the real data: 0 mismatches).
  * w is quantized host-side (tiny) to integers kw, laid out as
    lhsT [ci, tap, co] bf16 and duplicated into both partition halves.
  * conv = 9 shifted matmuls (K=ci=64, M=co=128) accumulating in PSUM.
    Integer products accumulate exactly in fp32 PSUM (|sum| <= 9.3e6 < 2^24).
    Two images run concurrently on the PE via row-tiling: image (2g) on
    partitions 0-63, image (2g+1) on partitions 64-127.
  * PSUM -> SBUF copy applies the final scale s2 = step_x*step_w and strips
    the padding columns; outputs DMA back per 16-row chunk.
"""

import os

import numpy as np
import ml_dtypes

import concourse.bass as bass
import concourse.mybir as mybir
import concourse.tile as tile
from concourse import bacc
from concourse.bass_utils import run_bass_kernel_spmd

dt = mybir.dt

N_CORES = 8
NPC = 4                # images per core
CI, CO = 64, 128
H = W = 56
WP = 58                # padded row width (56 + 2)
LEAD = 4               # guard elems before the padded grid
IMG_ELEMS = LEAD + WP * WP + 8   # 4 + 3364 + 8 = 3376
PACK = H * W           # 3136
MAGIC = 12582912.0     # 1.5 * 2^23 : fp32 round-to-nearest-even trick
CLIP = 127.25          # clip bound in scaled domain (exact in fp32)
H0S = [1 + 8 * i for i in range(7)]   # padded-row start of each 8-row block
BLK = 8 * WP           # 464 psum columns per block
N_WARM = 5             # PE warmup matmuls (HAM un-throttle)

_PROG_CACHE = {}


def _build_program(s_x, s2):
    """One SPMD program; per-core shards differ only through in_maps.

    s_x (=1/step_x) and s2 (=step_x*step_w) are embedded as immediates —
    the program is specialized per (alpha_x, alpha_w) value and cached.
    Immediates keep every instruction at <=1 semaphore wait (the TRN2
    TensorScalar ISA slot limit walrus enforces)."""
    s_x = float(np.float32(s_x))
    s2 = float(np.float32(s2))
    nc = bacc.Bacc(None)
    x_in = nc.declare_dram_parameter("x", [NPC * CI, PACK], dt.float32, isOutput=False)
    wq_in = nc.declare_dram_parameter("wq", [128, 9, CO], dt.bfloat16, isOutput=False)
    out = nc.declare_dram_parameter("out", [NPC * CO, PACK], dt.float32, isOutput=True)

    # quant chunks (data-row ranges) and the block groups they unlock.
    # First chunk is small so block 0's matmuls start as early as possible;
    # trailing single-block groups shrink the output-DMA tail.
    CHUNKS = [(0, 9), (9, 25), (25, 41), (41, 56)]
    ITERS = [[0], [1, 2], [3, 4], [5], [6]]

    with tile.TileContext(nc) as tc:
        with (
            tc.tile_pool(name="sb", bufs=1) as sb,
            tc.tile_pool(name="ps", bufs=4, space="PSUM") as psp,
        ):
            wq = sb.tile([128, 9, CO], dt.bfloat16)

            xs = [sb.tile([128, PACK], dt.float32, name=f"xs{g}", tag=f"xs{g}")
                  for g in range(2)]
            x2 = [sb.tile([128, PACK], dt.float32, name=f"x2{g}", tag=f"x2{g}")
                  for g in range(2)]
            xq = [sb.tile([128, IMG_ELEMS], dt.bfloat16, name=f"xq{g}", tag=f"xq{g}")
                  for g in range(2)]
            os_ = [sb.tile([128, PACK], dt.float32, name=f"os{n}", tag=f"os{n}")
                   for n in range(NPC)]

            wq_flat = wq.rearrange("p t c -> p (t c)")

            # input DMA, chunked; all on the SP ring, ordered so the first
            # quant chunk and then wq (for PE warmup) land earliest.
            def x_dma(g, ci):
                r0, r1 = CHUNKS[ci]
                nc.sync.dma_start(
                    out=xs[g][:, r0 * W:r1 * W],
                    in_=x_in[128 * g:128 * (g + 1), r0 * W:r1 * W])

            x_dma(0, 0)
            x_dma(0, 1)
            nc.sync.dma_start(out=wq[:], in_=wq_in[:])
            for ci in range(2, len(CHUNKS)):
                x_dma(0, ci)
            for ci in range(len(CHUNKS)):
                x_dma(1, ci)

            quant_mode = os.environ.get("KQ_MODE", "v2")
            if quant_mode == "v1":
                xp = [sb.tile([128, IMG_ELEMS], dt.float32, name=f"xp{g}",
                              tag=f"xp{g}") for g in range(2)]
                for g in range(2):
                    grid = xp[g][:, LEAD:LEAD + WP * WP].rearrange(
                        "p (r w) -> p r w", w=WP)
                    nc.vector.memset(xp[g][:, 0:LEAD + WP], MAGIC)
                    nc.vector.memset(grid[:, :, 0:1], MAGIC)
                    nc.vector.memset(grid[:, :, 57:58], MAGIC)
                    nc.vector.memset(xp[g][:, LEAD + 57 * WP:IMG_ELEMS], MAGIC)
                    nc.vector.tensor_scalar(
                        out=xs[g][:], in0=xs[g][:], scalar1=s_x, scalar2=CLIP,
                        op0=mybir.AluOpType.mult, op1=mybir.AluOpType.min)
                    nc.vector.tensor_scalar(
                        out=grid[:, 1:57, 1:57],
                        in0=xs[g][:].rearrange("p (r w) -> p r w", w=W),
                        scalar1=-CLIP, scalar2=MAGIC,
                        op0=mybir.AluOpType.max, op1=mybir.AluOpType.add)
                    nc.scalar.activation(
                        out=xq[g][:], in_=xp[g][:],
                        func=mybir.ActivationFunctionType.Copy,
                        bias=-MAGIC, scale=1.0)
            else:
                # zero the padded bf16 grids.  Full-tile memset (skinny
                # strided 16-bit border writes crash the runtime), on the
                # otherwise-idle GpSimd so the DVE queue isn't blocked.
                for g in range(2):
                    nc.gpsimd.memset(xq[g][:], 0.0)

                # quant pipeline, chunked:  P0/P1 on DVE (packed, 2x mode),
                # P2 on ACT does the pack -> padded-grid relayout.
                for g in range(2):
                    x23 = x2[g].rearrange("p (r w) -> p r w", w=W)
                    grid = xq[g][:, LEAD:LEAD + WP * WP].rearrange(
                        "p (r w) -> p r w", w=WP)
                    for (r0, r1) in CHUNKS:
                        cs = slice(r0 * W, r1 * W)
                        # P0: t = min(x*s, 127.25)   (in-place, packed)
                        nc.vector.tensor_scalar(
                            out=xs[g][:, cs], in0=xs[g][:, cs],
                            scalar1=s_x, scalar2=CLIP,
                            op0=mybir.AluOpType.mult, op1=mybir.AluOpType.min,
                        )
                        # P1: v = max(t, -127.25) + MAGIC   (packed)
                        nc.vector.tensor_scalar(
                            out=x2[g][:, cs], in0=xs[g][:, cs],
                            scalar1=-CLIP, scalar2=MAGIC,
                            op0=mybir.AluOpType.max, op1=mybir.AluOpType.add,
                        )
                        # P2: k = v - MAGIC -> bf16, into padded rows 1..56
                        nc.scalar.activation(
                            out=grid[:, 1 + r0:1 + r1, 1:57],
                            in_=x23[:, r0:r1, :],
                            func=mybir.ActivationFunctionType.Copy,
                            bias=-MAGIC, scale=1.0,
                        )

            for g in range(2):
                # 7 blocks of 8 output rows, processed in ITERS groups so
                # one PSUM tile spans <=2 banks; images 2g / 2g+1 concurrently
                # via PE row-tiling (partition halves).
                for blocks in ITERS:
                    b0, nb = blocks[0], len(blocks)
                    ps_pair = [psp.tile([128, 1024], dt.float32,
                                        name=f"psum_g{g}b{b0}h{h}", tag="ps")
                               for h in range(2)]
                    # each 464-wide block sits bank-aligned (cols 0 and 512)
                    ps2 = [p.rearrange("p (b x) -> p b x", b=2) for p in ps_pair]
                    if g == 0 and b0 == 0 and os.environ.get("KQ_WARM", "1") == "1":
                        # PE warmup (HAM un-throttle) while the quant head
                        # runs: garbage matmuls into a real psum tile; the
                        # first start=True tap matmul overwrites them.
                        for _ in range(N_WARM):
                            nc.tensor.matmul(
                                ps2[1][:, 0, 0:BLK], lhsT=wq[0:64, 0, :],
                                rhs=wq_flat[0:64, 0:BLK], start=True, stop=True,
                            )
                    for t in range(9):
                        dh, dw = t // 3, t % 3
                        for bi in range(nb):
                            off = LEAD + (H0S[b0 + bi] + dh - 1) * WP + (dw - 1)
                            # h=1 (ACT-freed slot / ACT-produced xq) first so
                            # PE's vector clock syncs on ACT before the h=0
                            # matmul, which then carries only its DVE wait
                            # (TRN2 matmul has a single sync-wait slot).
                            for h in (1, 0):
                                nc.tensor.matmul(
                                    ps2[h][:, bi, 0:BLK],
                                    lhsT=wq[64 * h:64 * (h + 1), t, :],
                                    rhs=xq[g][64 * h:64 * (h + 1), off:off + BLK],
                                    start=(t == 0), stop=(t == 8),
                                )
                    # scale + strip pad columns;  DVE for the even image,
                    # ACT for the odd one (balance the engines)
                    for h in range(2):
                        img = 2 * g + h
                        sel = ps2[h][:, 0:nb, 0:BLK].rearrange(
                            "p b (r w) -> p b r w", w=WP)[:, :, :, 1:57]
                        dst = os_[img].rearrange(
                            "p (b r w) -> p b r w", r=8, w=W)[:, b0:b0 + nb]
                        if h == 0:
                            nc.vector.tensor_scalar_mul(
                                out=dst, in0=sel, scalar1=s2)
                        else:
                            nc.scalar.activation(
                                out=dst, in_=sel,
                                func=mybir.ActivationFunctionType.Copy,
                                scale=s2,
                            )
                        nc.sync.dma_start(
                            out=out[CO * img:CO * (img + 1),
                                    448 * b0:448 * (b0 + nb)],
                            in_=os_[img][:, 448 * b0:448 * (b0 + nb)],
                        )
    if not nc.is_finalized():
        nc.finalize()   # Bacc: runs wait-splitting + register allocation
    return nc


def _host_prep(x, w, alpha_x, alpha_w):
    """Scalar/weight prep, replicating the reference's fp32 arithmetic."""
    x = np.ascontiguousarray(np.asarray(x, dtype=np.float32))
    w = np.asarray(w, dtype=np.float32)
    ax = np.float32(max(np.float32(np.asarray(alpha_x).reshape(-1)[0]), np.float32(0)))
    aw = np.float32(max(np.float32(np.asarray(alpha_w).reshape(-1)[0]), np.float32(0)))
    step_x = np.float32(np.float32(np.float32(2.0) * ax) / np.float32(254.0))
    step_w = np.float32(np.float32(np.float32(2.0) * aw) / np.float32(254.0))
    s_x = np.float32(np.float32(1.0) / step_x)
    s2 = np.float32(step_x * step_w)

    # weight quantization, integers in fp32 (exactly the reference math)
    kw = np.clip(np.round((w / step_w).astype(np.float32)), -127, 127)
    kw = kw.reshape(CO, CI, 9).transpose(1, 2, 0)          # [ci, tap, co]
    wq = np.concatenate([kw, kw], axis=0).astype(ml_dtypes.bfloat16)
    return x, wq, s_x, s2


def _in_maps(x, wq):
    return [
        {
            "x": x[NPC * c:NPC * (c + 1)].reshape(NPC * CI, PACK),
            "wq": wq,
        }
        for c in range(N_CORES)
    ]


def get_program(s_x=127.0, s2=float(np.float32(np.float32(1 / np.float32(127.0)) ** 2))):
    key = (float(np.float32(s_x)), float(np.float32(s2)))
    if key not in _PROG_CACHE:
        _PROG_CACHE[key] = _build_program(*key)
    return _PROG_CACHE[key]


def run_on_hw(x, w, alpha_x, alpha_w, trace=False):
    xx, wq, s_x, s2 = _host_prep(x, w, alpha_x, alpha_w)
    nc = get_program(s_x, s2)
    res = run_bass_kernel_spmd(nc, _in_maps(xx, wq),
                               list(range(N_CORES)), trace=trace)
    out = np.concatenate(
        [np.asarray(res.results[i]["out"]).reshape(NPC, CO, H, W)
         for i in range(N_CORES)], axis=0)
    return out.astype(np.float32, copy=False), res


def kernel(x, w, alpha_x, alpha_w):
    out, _ = run_on_hw(x, w, alpha_x, alpha_w)
    return out


# revision 37
# speedup vs baseline: 1.0080x; 1.0080x over previous
"""Quantized 3x3 conv (8-bit symmetric STE quantization of x and w, then
stride-1 pad-1 conv) on 8 Trainium2 NeuronCores.

Strategy
--------
Data-parallel over batch: 4 images per core (32/8).  Per core:
  * x is quantized on-device to integers kx in [-127,127] stored as bf16
    (exact), via 3 elementwise passes:
      P0 (DVE):  t = min(x * s, 127.25)            s = 1/step  (fp32)
      P1 (DVE):  v = max(t, -127.25) + 1.5*2^23    (magic round-half-even)
      P2 (ACT):  k = v - 1.5*2^23  -> bf16          (exact; relayout to a
                                                     58-wide zero-padded grid)
    This reproduces jnp.round(x/step) bit-exactly (verified vs the fp32
    reference on the real data: 0 mismatches).
  * w is quantized host-side (tiny) to integers kw, laid out as
    lhsT [ci, tap, co] bf16 and duplicated into both partition halves.
  * conv = 9 shifted matmuls (K=ci=64, M=co=128) accumulating in PSUM.
    Integer products accumulate exactly in fp32 PSUM (|sum| <= 9.3e6 < 2^24).
    Two images run concurrently on the PE via row-tiling: image (2g) on
    partitions 0-63, image (2g+1) on partitions 64-127.
  * PSUM -> SBUF copy applies the final scale s2 = step_x*step_w and strips
    the padding columns; outputs DMA back per 16-row chunk.
"""

import os

import numpy as np
import ml_dtypes

import concourse.bass as bass
import concourse.mybir as mybir
import concourse.tile as tile
from concourse import bacc
from concourse.bass_utils import run_bass_kernel_spmd

dt = mybir.dt

N_CORES = 8
NPC = 4                # images per core
CI, CO = 64, 128
H = W = 56
WP = 58                # padded row width (56 + 2)
LEAD = 4               # guard elems before the padded grid
IMG_ELEMS = LEAD + WP * WP + 8   # 4 + 3364 + 8 = 3376
PACK = H * W           # 3136
MAGIC = 12582912.0     # 1.5 * 2^23 : fp32 round-to-nearest-even trick
CLIP = 127.25          # clip bound in scaled domain (exact in fp32)
H0S = [1 + 8 * i for i in range(7)]   # padded-row start of each 8-row block
BLK = 8 * WP           # 464 psum columns per block
N_WARM = 8             # PE warmup matmuls (HAM un-throttle)

_PROG_CACHE = {}


def _build_program(s_x, s2):
    """One SPMD program; per-core shards differ only through in_maps.

    s_x (=1/step_x) and s2 (=step_x*step_w) are embedded as immediates —
    the program is specialized per (alpha_x, alpha_w) value and cached.
    Immediates keep every instruction at <=1 semaphore wait (the TRN2
    TensorScalar ISA slot limit walrus enforces)."""
    s_x = float(np.float32(s_x))
    s2 = float(np.float32(s2))
    nc = bacc.Bacc(None)
    x_in = nc.declare_dram_parameter("x", [NPC * CI, PACK], dt.float32, isOutput=False)
    wq_in = nc.declare_dram_parameter("wq", [128, 9, CO], dt.bfloat16, isOutput=False)
    out = nc.declare_dram_parameter("out", [NPC * CO, PACK], dt.float32, isOutput=True)

    # quant chunks (data-row ranges) and the block groups they unlock.
    # First chunk is small so block 0's matmuls start as early as possible;
    # trailing single-block groups shrink the output-DMA tail.
    CHUNKS = [(0, 9), (9, 25), (25, 41), (41, 56)]
    ITERS = [[0], [1, 2], [3, 4], [5], [6]]

    with tile.TileContext(nc) as tc:
        with (
            tc.tile_pool(name="sb", bufs=1) as sb,
            tc.tile_pool(name="ps", bufs=4, space="PSUM") as psp,
        ):
            wq = sb.tile([128, 9, CO], dt.bfloat16)

            xs = [sb.tile([128, PACK], dt.float32, name=f"xs{g}", tag=f"xs{g}")
                  for g in range(2)]
            x2 = [sb.tile([128, PACK], dt.float32, name=f"x2{g}", tag=f"x2{g}")
                  for g in range(2)]
            xq = [sb.tile([128, IMG_ELEMS], dt.bfloat16, name=f"xq{g}", tag=f"xq{g}")
                  for g in range(2)]
            os_ = [sb.tile([128, PACK], dt.float32, name=f"os{n}", tag=f"os{n}")
                   for n in range(NPC)]

            wq_flat = wq.rearrange("p t c -> p (t c)")

            # input DMA, chunked; all on the SP ring, ordered so the first
            # quant chunk and then wq (for PE warmup) land earliest.
            def x_dma(g, ci):
                r0, r1 = CHUNKS[ci]
                nc.sync.dma_start(
                    out=xs[g][:, r0 * W:r1 * W],
                    in_=x_in[128 * g:128 * (g + 1), r0 * W:r1 * W])

            nc.sync.dma_start(out=wq[:], in_=wq_in[:])
            x_dma(0, 0)
            x_dma(0, 1)
            for ci in range(2, len(CHUNKS)):
                x_dma(0, ci)
            for ci in range(len(CHUNKS)):
                x_dma(1, ci)

            quant_mode = os.environ.get("KQ_MODE", "v2")
            if quant_mode == "v1":
                xp = [sb.tile([128, IMG_ELEMS], dt.float32, name=f"xp{g}",
                              tag=f"xp{g}") for g in range(2)]
                for g in range(2):
                    grid = xp[g][:, LEAD:LEAD + WP * WP].rearrange(
                        "p (r w) -> p r w", w=WP)
                    nc.vector.memset(xp[g][:, 0:LEAD + WP], MAGIC)
                    nc.vector.memset(grid[:, :, 0:1], MAGIC)
                    nc.vector.memset(grid[:, :, 57:58], MAGIC)
                    nc.vector.memset(xp[g][:, LEAD + 57 * WP:IMG_ELEMS], MAGIC)
                    nc.vector.tensor_scalar(
                        out=xs[g][:], in0=xs[g][:], scalar1=s_x, scalar2=CLIP,
                        op0=mybir.AluOpType.mult, op1=mybir.AluOpType.min)
                    nc.vector.tensor_scalar(
                        out=grid[:, 1:57, 1:57],
                        in0=xs[g][:].rearrange("p (r w) -> p r w", w=W),
                        scalar1=-CLIP, scalar2=MAGIC,
                        op0=mybir.AluOpType.max, op1=mybir.AluOpType.add)
                    nc.scalar.activation(
                        out=xq[g][:], in_=xp[g][:],
                        func=mybir.ActivationFunctionType.Copy,
                        bias=-MAGIC, scale=1.0)
            else:
                # zero the padded bf16 grids.  Full-tile memset (skinny
                # strided 16-bit border writes crash the runtime), on the
                # otherwise-idle GpSimd so the DVE queue isn't blocked.
                for g in range(2):
                    nc.gpsimd.memset(xq[g][:], 0.0)

                # quant pipeline, chunked:  P0/P1 on DVE (packed, 2x mode),
                # P2 on ACT does the pack -> padded-grid relayout.
                for g in range(2):
                    x23 = x2[g].rearrange("p (r w) -> p r w", w=W)
                    grid = xq[g][:, LEAD:LEAD + WP * WP].rearrange(
                        "p (r w) -> p r w", w=WP)
                    for ci, (r0, r1) in enumerate(CHUNKS):
                        cs = slice(r0 * W, r1 * W)
                        # P0: t = min(x*s, 127.25)   (in-place, packed)
                        nc.vector.tensor_scalar(
                            out=xs[g][:, cs], in0=xs[g][:, cs],
                            scalar1=s_x, scalar2=CLIP,
                            op0=mybir.AluOpType.mult, op1=mybir.AluOpType.min,
                        )
                        # P1: v = max(t, -127.25) + MAGIC   (packed)
                        nc.vector.tensor_scalar(
                            out=x2[g][:, cs], in0=xs[g][:, cs],
                            scalar1=-CLIP, scalar2=MAGIC,
                            op0=mybir.AluOpType.max, op1=mybir.AluOpType.add,
                        )
                        # P2: k = v - MAGIC -> bf16, into padded rows 1..56.
                        # The very first chunk stays on DVE: no cross-engine
                        # hop on the head critical path.
                        if g == 0 and ci == 0:
                            nc.vector.tensor_scalar(
                                out=grid[:, 1 + r0:1 + r1, 1:57],
                                in0=x23[:, r0:r1, :],
                                scalar1=-MAGIC, scalar2=None,
                                op0=mybir.AluOpType.add,
                                op1=mybir.AluOpType.bypass,
                            )
                        else:
                            nc.scalar.activation(
                                out=grid[:, 1 + r0:1 + r1, 1:57],
                                in_=x23[:, r0:r1, :],
                                func=mybir.ActivationFunctionType.Copy,
                                bias=-MAGIC, scale=1.0,
                            )

            for g in range(2):
                # 7 blocks of 8 output rows, processed in ITERS groups so
                # one PSUM tile spans <=2 banks; images 2g / 2g+1 concurrently
                # via PE row-tiling (partition halves).
                for blocks in ITERS:
                    b0, nb = blocks[0], len(blocks)
                    ps_pair = [psp.tile([128, 1024], dt.float32,
                                        name=f"psum_g{g}b{b0}h{h}", tag="ps")
                               for h in range(2)]
                    # each 464-wide block sits bank-aligned (cols 0 and 512)
                    ps2 = [p.rearrange("p (b x) -> p b x", b=2) for p in ps_pair]
                    if g == 0 and b0 == 0 and os.environ.get("KQ_WARM", "1") == "1":
                        # PE warmup (HAM un-throttle) while the quant head
                        # runs: garbage matmuls into a real psum tile; the
                        # first start=True tap matmul overwrites them.
                        for _ in range(N_WARM):
                            nc.tensor.matmul(
                                ps2[1][:, 0, 0:BLK], lhsT=wq[0:64, 0, :],
                                rhs=wq_flat[0:64, 0:BLK], start=True, stop=True,
                            )
                    for t in range(9):
                        dh, dw = t // 3, t % 3
                        # h=1 (ACT-freed slot / ACT-produced xq) first so
                        # PE's vector clock syncs on ACT before the h=0
                        # matmuls, which then carry only their DVE wait
                        # (TRN2 matmul has a single sync-wait slot).
                        # h outer / bi inner: adjacent matmuls share weights.
                        for h in (1, 0):
                            for bi in range(nb):
                                off = LEAD + (H0S[b0 + bi] + dh - 1) * WP + (dw - 1)
                                nc.tensor.matmul(
                                    ps2[h][:, bi, 0:BLK],
                                    lhsT=wq[64 * h:64 * (h + 1), t, :],
                                    rhs=xq[g][64 * h:64 * (h + 1), off:off + BLK],
                                    start=(t == 0), stop=(t == 8),
                                )
                    # scale + strip pad columns;  DVE for the even image,
                    # ACT for the odd one (balance the engines)
                    for h in range(2):
                        img = 2 * g + h
                        sel = ps2[h][:, 0:nb, 0:BLK].rearrange(
                            "p b (r w) -> p b r w", w=WP)[:, :, :, 1:57]
                        dst = os_[img].rearrange(
                            "p (b r w) -> p b r w", r=8, w=W)[:, b0:b0 + nb]
                        if h == 0:
                            nc.vector.tensor_scalar_mul(
                                out=dst, in0=sel, scalar1=s2)
                        else:
                            nc.scalar.activation(
                                out=dst, in_=sel,
                                func=mybir.ActivationFunctionType.Copy,
                                scale=s2,
                            )
                        nc.sync.dma_start(
                            out=out[CO * img:CO * (img + 1),
                                    448 * b0:448 * (b0 + nb)],
                            in_=os_[img][:, 448 * b0:448 * (b0 + nb)],
                        )
    if not nc.is_finalized():
        nc.finalize()   # Bacc: runs wait-splitting + register allocation
    return nc


def _host_prep(x, w, alpha_x, alpha_w):
    """Scalar/weight prep, replicating the reference's fp32 arithmetic."""
    x = np.ascontiguousarray(np.asarray(x, dtype=np.float32))
    w = np.asarray(w, dtype=np.float32)
    ax = np.float32(max(np.float32(np.asarray(alpha_x).reshape(-1)[0]), np.float32(0)))
    aw = np.float32(max(np.float32(np.asarray(alpha_w).reshape(-1)[0]), np.float32(0)))
    step_x = np.float32(np.float32(np.float32(2.0) * ax) / np.float32(254.0))
    step_w = np.float32(np.float32(np.float32(2.0) * aw) / np.float32(254.0))
    s_x = np.float32(np.float32(1.0) / step_x)
    s2 = np.float32(step_x * step_w)

    # weight quantization, integers in fp32 (exactly the reference math)
    kw = np.clip(np.round((w / step_w).astype(np.float32)), -127, 127)
    kw = kw.reshape(CO, CI, 9).transpose(1, 2, 0)          # [ci, tap, co]
    wq = np.concatenate([kw, kw], axis=0).astype(ml_dtypes.bfloat16)
    return x, wq, s_x, s2


def _in_maps(x, wq):
    return [
        {
            "x": x[NPC * c:NPC * (c + 1)].reshape(NPC * CI, PACK),
            "wq": wq,
        }
        for c in range(N_CORES)
    ]


def get_program(s_x=127.0, s2=float(np.float32(np.float32(1 / np.float32(127.0)) ** 2))):
    key = (float(np.float32(s_x)), float(np.float32(s2)))
    if key not in _PROG_CACHE:
        _PROG_CACHE[key] = _build_program(*key)
    return _PROG_CACHE[key]


def run_on_hw(x, w, alpha_x, alpha_w, trace=False):
    xx, wq, s_x, s2 = _host_prep(x, w, alpha_x, alpha_w)
    nc = get_program(s_x, s2)
    res = run_bass_kernel_spmd(nc, _in_maps(xx, wq),
                               list(range(N_CORES)), trace=trace)
    out = np.concatenate(
        [np.asarray(res.results[i]["out"]).reshape(NPC, CO, H, W)
         for i in range(N_CORES)], axis=0)
    return out.astype(np.float32, copy=False), res


def kernel(x, w, alpha_x, alpha_w):
    out, _ = run_on_hw(x, w, alpha_x, alpha_w)
    return out
